# revision 42
# baseline (speedup 1.0000x reference)
"""Causal self-attention (B=2, T=2048, C=2048, NH=16) on 8 TRN2 NeuronCores.

Megatron-style tensor parallelism over heads: each core owns 2 heads.
All matmul operands are bf16 (fp32 PSUM accumulation): FWL halves the
LDWEIGHTS cost and every DMA moves half the bytes of the fp32 version.

Per core:
  phase 1: QKV projection in feature-major layout; q^T/k^T ([d, tokens])
           and V ([tokens, d]) are written straight into persistent SBUF
           tiles (no DRAM spill/reload).
  phase 2: causal attention per (batch, head) pair computed transposed:
           S^T[k,q] tiles = k^T_tile.T @ q^T_chunk, exp on ScalarE
           (PSUM->SBUF, bf16 out), 0/1 causal mask multiply on diagonal
           tiles, softmax denominator via an all-ones matmul
           (partition-dim sum), O^T[d,q] = V accumulation over k-tiles,
           reciprocal on DVE (custom approx op), multiply.
  phase 3: per-(batch, head, half-T) AllGather of y^T across cores
           (bf16), then each core computes its 256 output channels.
           out_proj(0) is emitted interleaved with attention pair (1,1)
           so its matmuls fill PE gaps; gathers fire per half as soon as
           the producing query chunks finish.
Host side: cast to bf16 + transpose/shard inputs, concat per-core output
column slices, cast back to fp32.
"""

import numpy as np
import ml_dtypes

import concourse.bacc as bacc
import concourse.mybir as mybir
import concourse.tile as tile
from concourse.bass_utils import run_bass_kernel_spmd

BF16 = mybir.dt.bfloat16
F32 = mybir.dt.float32
EXP = mybir.ActivationFunctionType.Exp

B, T, C, NH, HS = 2, 2048, 2048, 16, 128
NCORES = 8
HPC = NH // NCORES          # heads per core
BT = B * T                  # 4096 tokens total
CT = C // 128               # 16 contraction tiles
TCH = 512                   # phase-1 token chunk
NTCH = BT // TCH            # 8
Q = 512                     # phase-2 query chunk
NQC = T // Q                # 4 per (b, h)
EXPG = 2                    # k-tiles batched per exp instruction
P3CH = 256                  # phase-3 token chunk
OSL = C // NCORES           # 256 output channels per core


def build_nc(cc: bool = True):
    nc = bacc.Bacc("TRN2", target_bir_lowering=False, num_devices=NCORES)

    # inputs are host-blocked into exact SBUF tile layouts so each load is
    # 128 fat contiguous descriptors (dispatch cost is per-descriptor)
    xT = nc.dram_tensor("xT", [NTCH, 128, CT, TCH], BF16, kind="ExternalInput")
    wqkvT = nc.dram_tensor("wqkvT", [128, CT, 6 * HS], BF16, kind="ExternalInput")
    wprojT = nc.dram_tensor("wprojT", [128, CT, OSL], BF16, kind="ExternalInput")
    masks = nc.dram_tensor("masks", [128, 4, Q], BF16, kind="ExternalInput")
    ones = nc.dram_tensor("ones", [128, 128], BF16, kind="ExternalInput")
    out_loc = nc.dram_tensor("out_loc", [BT, OSL], BF16, kind="ExternalOutput")

    # y gathered per (batch, head, query-chunk): earlier collectives, more
    # overlap, and a short tail (the last out-proj chunks wait only on the
    # last 512 tokens). yg[b][hl][qf] rows are head-major with
    # head = 2*core + hl; the host permutes wprojT rows to match.
    y_loc = [
        [[nc.dram_tensor(f"y_loc{b}_{hl}_{qf}", [HS, Q], BF16) for qf in range(NQC)]
         for hl in range(HPC)]
        for b in range(B)
    ]
    yg = [
        [
            [
                nc.dram_tensor(
                    f"yg{b}_{hl}_{qf}", [NCORES * HS, Q], BF16,
                    addr_space="Shared" if cc else "Local",
                )
                for qf in range(NQC)
            ]
            for hl in range(HPC)
        ]
        for b in range(B)
    ]

    with tile.TileContext(nc) as tc:
        with (
            tc.tile_pool(name="const2", bufs=1) as const2,
            # persistent q/k/v tiles: phase 1 writes them, phase 2 reads them
            tc.tile_pool(name="qkv", bufs=1) as qkv_pool,
        ):
            masks_sb = const2.tile([128, 4, Q], BF16)
            nc.gpsimd.dma_start(out=masks_sb, in_=masks[:])
            ones_sb = const2.tile([128, 128], BF16)
            nc.gpsimd.dma_start(out=ones_sb, in_=ones[:])

            # layouts: q/k feature-major [d, hl, tok]; v token-major
            # [tok128, hl, ktile, d]
            q_res = [qkv_pool.tile([128, HPC, T], BF16, name=f"qres{b}") for b in range(B)]
            k_res = [qkv_pool.tile([128, HPC, T], BF16, name=f"kres{b}") for b in range(B)]
            v_res = [qkv_pool.tile([128, HPC, CT, HS], BF16, name=f"vres{b}") for b in range(B)]

            # ---------------- phase 1: QKV projection ----------------
            with (
                tc.tile_pool(name="wq", bufs=1) as wq_pool,
                tc.tile_pool(name="xin", bufs=2) as xin,
                tc.tile_pool(name="ps1", bufs=3, space="PSUM") as ps1,
                tc.tile_pool(name="psv", bufs=3, space="PSUM") as psv,
            ):
                wq_sb = wq_pool.tile([128, CT, 6 * HS], BF16)
                x_first = xin.tile([128, CT, TCH], BF16, name="x_sb")
                # interleave w/x loads, finest at the front, so the first
                # matmul (which needs only the 128 ot=0 columns of w-ctile 0
                # plus x-ctile 0) starts as early as possible
                nc.sync.dma_start(out=x_first[:, 0:1, :], in_=xT[0, :, 0:1, :])
                nc.sync.dma_start(out=wq_sb[:, 0, 0:128], in_=wqkvT[:, 0, 0:128])
                nc.sync.dma_start(
                    out=wq_sb[:, 0, 128 : 6 * HS], in_=wqkvT[:, 0, 128 : 6 * HS]
                )
                for lo, hi in ((1, 2), (2, 4), (4, 8), (8, 12), (12, 16)):
                    nc.sync.dma_start(
                        out=wq_sb[:, lo:hi, :], in_=wqkvT[:, lo:hi, :]
                    )
                    nc.sync.dma_start(
                        out=x_first[:, lo:hi, :], in_=xT[0, :, lo:hi, :]
                    )
                for tch in range(NTCH):
                    bb, tin = tch // (NTCH // B), (tch % (NTCH // B)) * TCH
                    tsl = slice(tin, tin + TCH)
                    if tch == 0:
                        x_sb = x_first
                    else:
                        x_sb = xin.tile([128, CT, TCH], BF16, name="x_sb")
                        for g in range(4):
                            nc.sync.dma_start(
                                out=x_sb[:, 4 * g : 4 * g + 4, :],
                                in_=xT[tch, :, 4 * g : 4 * g + 4, :],
                            )
                    for ot in range(4):  # q_h0, q_h1, k_h0, k_h1
                        pq = ps1.tile([128, TCH], F32)
                        for ci in range(CT):
                            nc.tensor.matmul(
                                pq[:],
                                wq_sb[:, ci, ot * 128 : (ot + 1) * 128],
                                x_sb[:, ci, :],
                                start=(ci == 0),
                                stop=(ci == CT - 1),
                            )
                        dst = (q_res if ot < 2 else k_res)[bb]
                        hl = ot % 2
                        nc.vector.tensor_copy(out=dst[:, hl, tsl], in_=pq[:])
                    for tt in range(TCH // 128):  # V in natural [token, d] layout
                        pv = psv.tile([128, 2 * HS], F32)
                        for ci in range(CT):
                            nc.tensor.matmul(
                                pv[:],
                                x_sb[:, ci, tt * 128 : (tt + 1) * 128],
                                wq_sb[:, ci, 4 * HS : 6 * HS],
                                start=(ci == 0),
                                stop=(ci == CT - 1),
                            )
                        ktg = (tin + tt * 128) // 128
                        nc.vector.tensor_copy(
                            out=v_res[bb][:, :, ktg, :], in_=pv[:]
                        )

            # ---------------- phases 2+3: attention, gather, out-proj ----------------
            with (
                tc.tile_pool(name="esp", bufs=2) as esp,
                tc.tile_pool(name="es2p", bufs=2) as es2p,
                tc.tile_pool(name="es4p", bufs=2) as es4p,
                tc.tile_pool(name="rp", bufs=2) as rp,
                tc.tile_pool(name="yst", bufs=2) as yst,
                tc.tile_pool(name="wp", bufs=1) as wp_pool,
                tc.tile_pool(name="ygp", bufs=5) as ygp,
                tc.tile_pool(name="ost", bufs=2) as ost,
                tc.tile_pool(name="ps_s", bufs=2, space="PSUM") as ps_s,
                tc.tile_pool(name="ps_d", bufs=1, space="PSUM") as ps_d,
                tc.tile_pool(name="ps_o", bufs=1, space="PSUM") as ps_o,
                tc.tile_pool(name="ps3", bufs=2, space="PSUM") as ps3,
            ):
                wp_sb = wp_pool.tile([128, CT, OSL], BF16)
                nc.gpsimd.dma_start(out=wp_sb, in_=wprojT[:])

                # denom/AV matmuls are emitted one chunk late so the in-order
                # PE queue has S-matmuls of the next chunk to chew on while
                # the last exp group of the current chunk drains via ACT/DVE
                pending: list = []

                def flush_pending():
                    while pending:
                        pending.pop(0)()

                def denom_av(b, hl, qc, nk, es, es4):
                    # denominator over the twice-DVE-paired tiles: 1/4 the
                    # PE streaming of a full ones @ es reduction
                    dp = ps_d.tile([128, Q], F32, name="dp")
                    for kt in range(nk // 4):
                        nc.tensor.matmul(
                            dp[:], ones_sb[:], es4[:, kt, :],
                            start=(kt == 0), stop=(kt == nk // 4 - 1),
                        )
                    # 1/x on DVE: custom approx op (~51 ULP), far cheaper
                    # than the 2-pass exp(-ln(x)) on ScalarE
                    r_sb = rp.tile([128, Q], F32, name="r_sb")
                    nc.vector.reciprocal_approx_fast(out=r_sb[:], in_=dp[:])
                    po = ps_o.tile([128, Q], F32, name="po")
                    for kt in range(nk):
                        # diagonal k-tiles: q-columns below the diagonal are
                        # exactly zero in es, skip streaming them (kt=0 is
                        # always full-width, so start=True covers the bank)
                        a = kt - (nk - 4)
                        off = a * 128 if a > 0 else 0
                        nc.tensor.matmul(
                            po[:, off:], v_res[b][:, hl, kt, :], es[:, kt, off:],
                            start=(kt == 0), stop=(kt == nk - 1),
                        )
                    y_sb = yst.tile([128, Q], BF16, name="y_sb")
                    nc.vector.tensor_mul(out=y_sb[:], in0=po[:], in1=r_sb[:])
                    nc.sync.dma_start(out=y_loc[b][hl][qc][:], in_=y_sb[:])

                def attention_chunk(b: int, hl: int, qc: int):
                    nk = (qc + 1) * (Q // 128)  # causal: k-tiles 0..nk-1
                    qsl = slice(qc * Q, (qc + 1) * Q)
                    es = esp.tile([128, CT, Q], BF16, name="es")
                    es2 = es2p.tile([128, CT // 2, Q], BF16, name="es2")
                    es4 = es4p.tile([128, CT // 4, Q], BF16, name="es4")
                    # the very first chunk must write its PSUM slots full-width
                    # so later trimmed writes leave stale-but-finite data (the
                    # full-width exp of a trimmed region reads the previous
                    # group's finite scores; the mask multiply zeroes it)
                    first_chunk = b == 0 and hl == 0 and qc == 0
                    for g in range(nk // EXPG):
                        sp = ps_s.tile([128, EXPG * Q], F32, name="sp")
                        for j in range(EXPG):
                            kt = g * EXPG + j
                            a = kt - (nk - 4)
                            off = a * 128 if (a > 0 and not first_chunk) else 0
                            nc.tensor.matmul(
                                sp[:, j * Q + off : (j + 1) * Q],
                                k_res[b][:, hl, kt * 128 : (kt + 1) * 128],
                                q_res[b][:, hl, qc * Q + off : (qc + 1) * Q],
                                start=True,
                                stop=True,
                            )
                        # trim the exp to the group's widest valid region;
                        # stale values left of it are zeroed by the mask
                        # (the first two chunks are the first use of each es
                        # pool slot: write them full so stale reads later are
                        # finite, never uninitialized bits)
                        ag = 2 * g - (nk - 4)
                        first_es = b == 0 and hl == 0 and qc <= 1
                        goff = ag * 128 if (ag > 0 and not first_es) else 0
                        nc.scalar.activation(
                            out=es[:, g * EXPG : (g + 1) * EXPG, goff:],
                            in_=sp[:].rearrange("p (a q) -> p a q", a=EXPG)[
                                :, :, goff:
                            ],
                            func=EXP,
                        )
                        if g * EXPG >= nk - 4:  # diagonal groups -> 0/1 mask
                            a0 = g * EXPG - (nk - 4)
                            nc.vector.tensor_tensor(
                                es[:, g * EXPG : (g + 1) * EXPG, :],
                                es[:, g * EXPG : (g + 1) * EXPG, :],
                                masks_sb[:, a0 : a0 + EXPG, :],
                                mybir.AluOpType.mult,
                            )
                        # pair-sum the two exp tiles (post-mask) for the
                        # denominator (EXPG == 2), then pair the pairs
                        nc.vector.tensor_tensor(
                            es2[:, g, :],
                            es[:, 2 * g, :],
                            es[:, 2 * g + 1, :],
                            mybir.AluOpType.add,
                        )
                        if g % 2 == 1:
                            nc.vector.tensor_tensor(
                                es4[:, g // 2, :],
                                es2[:, g - 1, :],
                                es2[:, g, :],
                                mybir.AluOpType.add,
                            )
                    flush_pending()
                    pending.append(
                        lambda b=b, hl=hl, qc=qc, nk=nk, es=es, es4=es4: denom_av(
                            b, hl, qc, nk, es, es4
                        )
                    )

                def gather(b: int, hl: int, qf: int):
                    if cc:
                        nc.gpsimd.collective_compute(
                            "AllGather",
                            mybir.AluOpType.bypass,
                            replica_groups=[list(range(NCORES))],
                            ins=[y_loc[b][hl][qf].ap()],
                            outs=[yg[b][hl][qf].ap()],
                        )
                    else:  # timing-only variant: no inter-core traffic
                        nc.sync.dma_start(
                            out=yg[b][hl][qf][:HS, :], in_=y_loc[b][hl][qf].ap()
                        )

                def out_proj_chunk(b: int, ch: int):
                    # contract over even-head gather rows then odd-head rows;
                    # wprojT rows are host-permuted to match
                    qf = (ch * P3CH) // Q
                    csl = slice(ch * P3CH - qf * Q, (ch + 1) * P3CH - qf * Q)
                    yg_sb = ygp.tile([128, CT, P3CH], BF16, name="yg_sb")
                    for hl in range(HPC):
                        # split dispatch across two DMA paths
                        eng = nc.gpsimd if hl == 0 else nc.sync
                        eng.dma_start(
                            out=yg_sb[:, hl * (CT // 2) : (hl + 1) * (CT // 2), :],
                            in_=yg[b][hl][qf][:, csl].rearrange(
                                "(ko p) t -> p ko t", p=128
                            ),
                        )
                    for tt in range(P3CH // 128):
                        po = ps3.tile([128, OSL], F32, name="po3")
                        for ci in range(CT):
                            nc.tensor.matmul(
                                po[:],
                                yg_sb[:, ci, tt * 128 : (tt + 1) * 128],
                                wp_sb[:, ci, :],
                                start=(ci == 0),
                                stop=(ci == CT - 1),
                            )
                        o_sb = ost.tile([128, OSL], BF16, name="o_sb")
                        nc.vector.tensor_copy(out=o_sb[:], in_=po[:])
                        nc.sync.dma_start(
                            out=out_loc[
                                b * T + ch * P3CH + tt * 128 : b * T
                                + ch * P3CH
                                + (tt + 1) * 128,
                                :,
                            ],
                            in_=o_sb[:],
                        )

                # gathers fire per query-chunk as soon as the producing
                # denom_av is emitted (one chunk late via `pending`);
                # out_proj chunks interleave into the later attention pairs,
                # lagging their gathers by a few chunks so their yg loads
                # never head-of-line-block the in-order PE queue
                attention_chunk(0, 0, 0)
                attention_chunk(0, 0, 1)
                gather(0, 0, 0)
                attention_chunk(0, 0, 2)
                gather(0, 0, 1)
                attention_chunk(0, 0, 3)
                gather(0, 0, 2)
                attention_chunk(0, 1, 0)   # start flushed (0,0,3)
                gather(0, 0, 3)
                attention_chunk(0, 1, 1)
                gather(0, 1, 0)
                attention_chunk(0, 1, 2)
                gather(0, 1, 1)
                attention_chunk(0, 1, 3)
                gather(0, 1, 2)
                attention_chunk(1, 0, 0)   # start flushed (0,1,3)
                gather(0, 1, 3)
                out_proj_chunk(0, 0)
                attention_chunk(1, 0, 1)
                gather(1, 0, 0)
                out_proj_chunk(0, 1)
                out_proj_chunk(0, 2)
                attention_chunk(1, 0, 2)
                gather(1, 0, 1)
                out_proj_chunk(0, 3)
                out_proj_chunk(0, 4)
                attention_chunk(1, 0, 3)
                gather(1, 0, 2)
                out_proj_chunk(0, 5)
                attention_chunk(1, 1, 0)   # start flushed (1,0,3)
                gather(1, 0, 3)
                out_proj_chunk(0, 6)
                attention_chunk(1, 1, 1)
                gather(1, 1, 0)
                out_proj_chunk(0, 7)
                attention_chunk(1, 1, 2)
                gather(1, 1, 1)
                out_proj_chunk(1, 0)
                out_proj_chunk(1, 1)
                out_proj_chunk(1, 2)
                attention_chunk(1, 1, 3)
                gather(1, 1, 2)
                out_proj_chunk(1, 3)
                out_proj_chunk(1, 4)
                out_proj_chunk(1, 5)
                flush_pending()
                gather(1, 1, 3)
                out_proj_chunk(1, 6)
                out_proj_chunk(1, 7)

    nc.finalize()
    return nc


def prep_inputs(x: np.ndarray, w_attn: np.ndarray, w_proj: np.ndarray):
    """Host-side sharding/layout. Returns per-core input maps."""
    bf = ml_dtypes.bfloat16
    # blocked to [chunk, partition, c-tile, token]
    xT = np.ascontiguousarray(
        x.reshape(NTCH, TCH, CT, 128).transpose(0, 3, 2, 1)
    ).astype(bf)
    wq, wk, wv = w_attn[:C], w_attn[C : 2 * C], w_attn[2 * C :]
    scale = np.float32(1.0 / np.sqrt(HS))
    kk = np.arange(128, dtype=np.int64)[:, None, None]
    aa = np.arange(4, dtype=np.int64)[None, :, None]
    qq = np.arange(Q, dtype=np.int64)[None, None, :]
    masks = (128 * aa + kk <= qq).astype(bf)
    in_maps = []
    for c in range(NCORES):
        h0 = HPC * c
        rows = slice(h0 * HS, (h0 + HPC) * HS)
        wqkvT = np.ascontiguousarray(
            np.concatenate([wq[rows] * scale, wk[rows], wv[rows]], axis=0)
            .T.reshape(CT, 128, 6 * HS)
            .transpose(1, 0, 2)
        ).astype(bf)
        # rows permuted to the per-(batch,head) gather layout: the gathers
        # concatenate cores, so channel order is even heads (hl=0) then odd
        # heads (hl=1), head = 2*core + hl
        perm = np.concatenate(
            [
                np.arange(HS) + h * HS
                for hl in range(HPC)
                for h in range(hl, NH, HPC)
            ]
        )
        wprojT = np.ascontiguousarray(
            w_proj[c * OSL : (c + 1) * OSL, perm]
            .T.reshape(CT, 128, OSL)
            .transpose(1, 0, 2)
        ).astype(bf)
        in_maps.append(
            {
                "xT": xT,
                "wqkvT": wqkvT,
                "wprojT": wprojT,
                "masks": masks,
                "ones": np.ones((128, 128), dtype=bf),
            }
        )
    return in_maps


_CACHE: dict = {}


def _get_nc(cc: bool = True):
    key = ("nc", cc)
    if key not in _CACHE:
        _CACHE[key] = build_nc(cc=cc)
    return _CACHE[key]


def run(x, w_attn, w_proj, cc: bool = True, **spmd_kwargs):
    nc = _get_nc(cc=cc)
    in_maps = prep_inputs(
        np.asarray(x, dtype=np.float32),
        np.asarray(w_attn, dtype=np.float32),
        np.asarray(w_proj, dtype=np.float32),
    )
    res = run_bass_kernel_spmd(nc, in_maps, list(range(NCORES)), **spmd_kwargs)
    out = np.concatenate(
        [np.asarray(res.results[c]["out_loc"]) for c in range(NCORES)], axis=1
    ).astype(np.float32)
    return out.reshape(B, T, C), res


def kernel(x, w_attn, w_proj):
    out, _ = run(x, w_attn, w_proj, cc=True)
    return out
